# revision 13
# baseline (speedup 1.0000x reference)
"""Linear-chain CRF loss (forward partition + gold score) on 8 Trainium2 cores.

Strategy (v4, compacted chunked matrix scan):
  The 512-step forward recurrence p' = diag(e_t) @ expT.T @ p (exp space) is
  linear, so alpha at any position is a product of per-step 32x32 factors.
  Sequences are RIGHT-ALIGNED on the host: a sequence of length n gets
  S - n leading "pad" steps whose exp-space emission is onehot(j0); a pad
  step maps onehot(j0) -> kappa * onehot(j0) (kappa = exp(T[j0,j0])), a
  known scalar correction. The first real step's emission is pre-adjusted
  by expT[START,:]/expT[j0,:] so the chain seeded with onehot(j0)
  reproduces the true alpha recurrence, and every sequence's needed state
  (alpha at len-1) is the final state.

  The time axis is cut into C = 64 chunks of K = 8 steps. Chunks made
  entirely of pads act as known scalars on onehot(j0) and are skipped.
  Each remaining (seq, chunk) pair is an independent "cell" computing the
  8-step 32x32 chunk-product matrix. The host bin-packs all real cells
  (~2200 of 4096) onto 8 cores x 288 device cells; overflow (rare) is
  computed on the host in fp64. Cells pack 4-to-a-partition-block (lane
  l = 0..3) x free groups; one 128x128 block-diagonal expT weight serves
  every matmul. Per step: one matmul (bf16, N<=512) + one PSUM->SBUF move
  (ACT copy or DVE-fused multiply) + one broadcast emission multiply (DVE
  bf16 2x), batched across 32+ chunk-slots. Four asymmetric batches
  ([3,2,2,2] x 256 columns) pipeline across PE/ACT/DVE. Step 0 of each
  cell is diag(e) @ T' (pure input data) computed by one DVE multiply from
  a broadcast T' tile; the final step's emission multiply is deferred to
  the host (the device ships the raw step-7 matmul result), which also
  shortens the output tail. The host combines each sequence's chunk
  matrices in order (fp64 matvecs), applies the kappa correction and final
  logsumexp; the gold-path score is tiny table gathers on host.
"""

import numpy as np
import ml_dtypes

START_IDX = 29
END_IDX = 30
PAD_IDX = 31

B, S, L = 64, 512, 32
NCORES = 8
J0 = 0              # anchor state for pad steps
KK = 8              # steps per chunk
C = S // KK         # chunks per sequence
NB = 4              # batch pipelines per core
NGB = (3, 2, 2, 2)  # g_hi count per batch
OFF = (0, 3, 5, 7)  # prefix offsets of NGB
NGH = sum(NGB)                    # 9
NCD = 4 * NGH                     # device chunk-slots: c_dev in [0, 36)
CELLS = NCD * 8                   # 288 cells per core
EMCOLS = 8 * KK * NCD             # 2304
BF = ml_dtypes.bfloat16

# (h, s) layer-batches whose PSUM read fuses into the DVE multiply
# (1x from PSUM) instead of ACT copy + 2x DVE multiply; balances ACT vs DVE.
FUSED = {(0, 2), (0, 5), (1, 1), (1, 4), (2, 3), (2, 6), (3, 2), (3, 5)}

_nc = None


def _build_nc():
    import concourse.bacc as bacc
    import concourse.bass as bass
    import concourse.mybir as mybir
    from concourse import tile

    bf = mybir.dt.bfloat16
    f32 = mybir.dt.float32
    nc = bacc.Bacc(None, target_bir_lowering=False)

    inp_in = nc.declare_dram_parameter("inp", (128, 384 + EMCOLS), bf,
                                       isOutput=False)
    am_out = nc.declare_dram_parameter("am", (128, EMCOLS), bf, isOutput=True)

    with tile.TileContext(nc) as tc:
        with (
            tc.tile_pool(name="st", bufs=1) as st,
            tc.tile_pool(name="ps", bufs=1, space=bass.MemorySpace.PSUM) as psp,
        ):
            WEM0 = st.tile([128, 1152], bf)      # [wt(384) | em batch0(768)]
            EMR = st.tile([128, 1536], bf)       # em batches 1..3
            X = [st.tile([128, 256 * NGB[h]], bf, name=f"X{h}")
                 for h in range(NB)]
            SS = [st.tile([128, 256 * NGB[h]], bf, name=f"SS{h}")
                  for h in range(NB)]
            SSA = st.tile([128, 1280], bf)       # final states, batches 0-1
            SSB = st.tile([128, 1024], bf)       # final states, batches 2-3
            PS = [psp.tile([128, 256 * NGB[h]], f32, name=f"PS{h}")
                  for h in range(NB)]

            nc.sync.dma_start(WEM0[:], inp_in[:, 0:1152])
            nc.scalar.dma_start(EMR[:], inp_in[:, 1152:2688])

            WT = WEM0[:, 0:384]
            TBD = WEM0[:, 0:128]
            # TP8[32l+i, j*8+gl] = T'[i, j], replicated over lanes and gl
            em_src = [WEM0[:, 384:1152], EMR[:, 0:512], EMR[:, 512:1024],
                      EMR[:, 1024:1536]]
            em3 = [
                em_src[h].rearrange("p (blk c) -> p blk c", c=256)
                for h in range(NB)
            ]

            def em_view(h, s):
                v = em3[h][:, 0:NGB[h], 8 * s:8 * s + 8]  # (128, NGB, 8)
                return v[:, None, :, :].broadcast_to([128, 32, NGB[h], 8])

            def tp_view(h):
                v = WT[:, 128:384].rearrange("p (j gl) -> p j gl", j=32)
                return v[:, :, None, :].broadcast_to([128, 32, NGB[h], 8])

            x3 = [
                X[h][:].rearrange("p (j gh gl) -> p j gh gl", j=32, gh=NGB[h])
                for h in range(NB)
            ]
            s3 = [
                SS[h][:].rearrange("p (j gh gl) -> p j gh gl", j=32, gh=NGB[h])
                for h in range(NB)
            ]
            p3 = [
                PS[h][:].rearrange("p (j gh gl) -> p j gh gl", j=32, gh=NGB[h])
                for h in range(NB)
            ]

            # layer 0: A_1 = diag(e_0) @ T'  (chain seeds at I)
            for h in range(NB):
                nc.vector.tensor_mul(x3[h], tp_view(h), em_view(h, 0))
            for s in range(1, KK):
                for h in range(NB):
                    fd = 256 * NGB[h]
                    for n0 in range(0, fd, 512):
                        n1 = min(n0 + 512, fd)
                        nc.tensor.matmul(
                            PS[h][:, n0:n1], TBD, X[h][:, n0:n1],
                            start=True, stop=True,
                        )
                    if s == KK - 1:
                        # final move only; host applies the last emission.
                        # batches 0/2 move on ACT, 1/3 on DVE (tail overlap)
                        fin = {0: SSA[:, 0:768], 1: SSA[:, 768:1280],
                               2: SSB[:, 0:512], 3: SSB[:, 512:1024]}[h]
                        if h % 2 == 0:
                            nc.scalar.copy(fin, PS[h][:])
                        else:
                            nc.vector.tensor_copy(fin, PS[h][:])
                        if h == 1:
                            nc.sync.dma_start(am_out[:, 0:1280], SSA[:])
                        elif h == 3:
                            nc.scalar.dma_start(am_out[:, 1280:2304], SSB[:])
                    elif (h, s) in FUSED:
                        nc.vector.tensor_mul(x3[h], p3[h], em_view(h, s))
                    else:
                        nc.scalar.copy(SS[h][:], PS[h][:])
                        nc.vector.tensor_mul(x3[h], s3[h], em_view(h, s))

    nc.compile()
    return nc


def _labeled_score(lstm_scores, word_seq_lens, tags, mask, transition):
    b_idx = np.arange(B)
    t0 = tags[:, 0]
    begin = transition[START_IDX, t0].astype(np.float64) + lstm_scores[b_idx, 0, t0]
    prev, curt = tags[:, :-1], tags[:, 1:]
    trans_mid = transition[prev, curt].astype(np.float64)
    em_mid = np.take_along_axis(lstm_scores[:, 1:, :], curt[..., None], axis=2)[..., 0]
    mid = np.where(mask[:, 1:], trans_mid + em_mid, 0.0)
    end_ids = tags[b_idx, word_seq_lens - 1]
    end_sc = transition[end_ids, END_IDX].astype(np.float64)
    return begin.sum() + end_sc.sum() + mid.sum()


def _plan(word_seq_lens):
    """Assign sequences to cores and (seq, chunk) pairs to device cells."""
    lens = word_seq_lens.astype(np.int64)
    shifts = S - lens
    skips = shifts // KK                # leading all-pad chunks
    nreal = C - skips                   # non-pad chunks per seq
    order = np.argsort(-nreal, kind="stable")
    core_seqs = [[] for _ in range(NCORES)]
    sums = np.zeros(NCORES, dtype=np.int64)
    for b in order:
        cg = int(np.argmin(sums))
        core_seqs[cg].append(int(b))
        sums[cg] += nreal[b]
    core_cells = []
    overflow = []
    for cg in range(NCORES):
        cells = []
        for b in core_seqs[cg]:
            for k in range(int(nreal[b])):
                if len(cells) < CELLS:
                    cells.append((b, k))
                else:
                    overflow.append((b, k))
        core_cells.append(cells)
    return core_cells, skips, nreal, overflow


def _prep_inputs(lstm_scores, word_seq_lens, transition):
    """Exp-space right-aligned emissions, packed into device cells (bf16)."""
    T = transition.astype(np.float64)
    expT = np.exp(T)
    Tp = expT.T  # T'[i, j] = expT[j, i]

    lens = word_seq_lens.astype(np.int64)
    shifts = S - lens
    EMfull = np.zeros((B, S, L))
    tau = np.arange(S)[None, :]
    EMfull[:, :, J0] = np.where(tau < shifts[:, None], 1.0, 0.0)
    e0 = np.exp(lstm_scores[:, 0, :].astype(np.float64) + T[START_IDX] - T[J0])
    e0[:, START_IDX] = 0.0
    e0[:, PAD_IDX] = 0.0
    real = np.exp(lstm_scores.astype(np.float64))
    for b in range(B):
        sh = int(shifts[b])
        EMfull[b, sh] = e0[b]
        if sh + 1 < S:
            EMfull[b, sh + 1:] = real[b, 1:S - sh]

    wt = np.zeros((128, 384))
    for l in range(4):
        wt[32 * l:32 * l + 32, 32 * l:32 * l + 32] = expT
    wt[:, 128:384] = np.tile(np.repeat(Tp, 8, axis=1), (4, 1))
    wt = wt.astype(BF)

    core_cells, skips, nreal, overflow = _plan(word_seq_lens)

    ems = []
    e7s = []    # per-core last-step emissions per cell, fp64 [NCD, 8, L]
    for cg in range(NCORES):
        cells = core_cells[cg]
        B_idx = np.zeros((NCD, 8), dtype=np.int64)
        Ct_idx = np.zeros((NCD, 8), dtype=np.int64)
        dummy = np.ones((NCD, 8), dtype=bool)
        for e, (b, k) in enumerate(cells):
            cd, sl = e // 8, e % 8
            B_idx[cd, sl] = b
            Ct_idx[cd, sl] = skips[b] + k
            dummy[cd, sl] = False
        tsel = (KK * Ct_idx[:, None, :] + np.arange(KK)[None, :, None])
        g = EMfull[B_idx[:, None, :], tsel, :]        # (NCD, KK, 8, L)
        oh = np.zeros(L)
        oh[J0] = 1.0
        g[dummy[:, None, :].repeat(KK, 1)] = oh
        e7s.append(g[:, KK - 1, :, :].copy())         # (NCD, 8, L)
        em3d = np.ascontiguousarray(g.transpose(3, 0, 1, 2)).reshape(
            L, NCD * KK * 8)                          # [i, 8*(KK*cd+s)+slot]
        em_rep = np.zeros((128, EMCOLS))
        for l in range(4):
            w = EMCOLS - 64 * l
            em_rep[32 * l:32 * l + 32, :w] = em3d[:, 64 * l:]
        ems.append(np.concatenate(
            [wt.astype(np.float64), em_rep], axis=1).astype(BF))

    return ems, wt, core_cells, skips, nreal, overflow, EMfull, Tp, e7s


def _device_emulate(ems, wt):
    """Numpy emulation of the device kernel (bf16), for validation."""
    outs = []
    tp8 = wt[:, 128:384].astype(np.float64)
    for cg in range(NCORES):
        em = ems[cg].astype(np.float64)[:, 384:]
        res = np.zeros((128, EMCOLS))
        for h in range(NB):
            lo, hi = 256 * OFF[h], 256 * (OFF[h] + NGB[h])
            EMh = em[:, lo:hi]
            ngb = NGB[h]

            def emul(s):
                ev = EMh.reshape(128, ngb, 256)[:, :, 8 * s:8 * s + 8]
                return np.repeat(ev[:, None, :, :], 32, axis=1).reshape(128, -1)

            tpv = np.repeat(
                tp8.reshape(128, 32, 8)[:, :, None, :], ngb, axis=2
            ).reshape(128, -1)
            Xh = np.asarray(tpv * emul(0), dtype=BF).astype(np.float64)
            for s in range(1, KK):
                q = np.zeros_like(Xh)
                for l in range(4):
                    tl = wt[32 * l:32 * l + 32, 32 * l:32 * l + 32] \
                        .astype(np.float64)
                    q[32 * l:32 * l + 32] = tl.T @ Xh[32 * l:32 * l + 32]
                if s == KK - 1:
                    Xh = np.asarray(q, dtype=BF).astype(np.float64)
                else:
                    Xh = np.asarray(q * emul(s), dtype=BF).astype(np.float64)
            res[:, lo:hi] = Xh
        outs.append(res)
    return outs


def _combine(am_list, core_cells, skips, nreal, overflow, EMfull, Tp, e7s,
             word_seq_lens, transition):
    T = transition.astype(np.float64)
    t_end = T[:, END_IDX]
    kappa = T[J0, J0]
    lens = word_seq_lens.astype(np.int64)

    Aof = {}
    for cg in range(NCORES):
        am = np.asarray(am_list[cg]).astype(np.float64)
        e7 = e7s[cg]
        for e, (b, k) in enumerate(core_cells[cg]):
            cd, sl = e // 8, e % 8
            l, ghg = cd % 4, cd // 4
            h = next(hh for hh in range(NB)
                     if OFF[hh] <= ghg < OFF[hh] + NGB[hh])
            gh = ghg - OFF[h]
            cols = (256 * OFF[h] + np.arange(32) * 8 * NGB[h] + gh * 8 + sl)
            # device shipped raw step-7 matmul result; apply last emission
            Aof[(b, k)] = am[32 * l:32 * l + 32][:, cols] * e7[cd, sl][:, None]
    for (b, k) in overflow:
        c_real = int(skips[b] + k)
        M = EMfull[b, KK * c_real][:, None] * Tp
        for s in range(1, KK):
            M = EMfull[b, KK * c_real + s][:, None] * (Tp @ M)
        Aof[(b, k)] = M

    unlabeled = 0.0
    for b in range(B):
        v = np.zeros(L)
        v[J0] = 1.0
        logacc = 0.0
        for k in range(int(nreal[b])):
            v = Aof[(b, k)] @ v
            m = v.max()
            logacc += np.log(m)
            v = v / m
        sh = int(S - lens[b] - skips[b] * KK)
        with np.errstate(divide="ignore"):
            la = np.log(v) + logacc - sh * kappa + t_end
        mm = la.max()
        unlabeled += mm + np.log(np.exp(la - mm).sum())
    return unlabeled


def kernel(lstm_scores, word_seq_lens, tags, mask, transition, _emulate=False):
    global _nc
    lstm_scores = np.asarray(lstm_scores, dtype=np.float32)
    word_seq_lens = np.asarray(word_seq_lens).astype(np.int64)
    tags = np.asarray(tags).astype(np.int64)
    mask = np.asarray(mask).astype(bool)
    transition = np.asarray(transition, dtype=np.float32)

    ems, wt, core_cells, skips, nreal, overflow, EMfull, Tp, e7s = \
        _prep_inputs(lstm_scores, word_seq_lens, transition)

    if _emulate:
        am_list = _device_emulate(ems, wt)
    else:
        if _nc is None:
            _nc = _build_nc()
        in_maps = [{"inp": ems[cg]} for cg in range(NCORES)]
        from concourse.bass_utils import run_bass_kernel_spmd
        res = run_bass_kernel_spmd(_nc, in_maps, list(range(NCORES)))
        am_list = [res.results[cg]["am"] for cg in range(NCORES)]

    unlabeled = _combine(am_list, core_cells, skips, nreal, overflow,
                         EMfull, Tp, e7s, word_seq_lens, transition)
    labeled = _labeled_score(lstm_scores, word_seq_lens, tags, mask, transition)
    return (np.float32(unlabeled), np.float32(labeled))


# revision 20
# speedup vs baseline: 1.0421x; 1.0421x over previous
"""Linear-chain CRF loss (forward partition + gold score) on 8 Trainium2 cores.

Strategy (compacted chunked matrix scan):
  The 512-step forward recurrence p' = diag(e_t) @ expT.T @ p (exp space) is
  linear, so alpha at any position is a product of per-step 32x32 factors.
  Sequences are RIGHT-ALIGNED on the host: a sequence of length n gets
  S - n leading "pad" steps whose exp-space emission is onehot(j0); a pad
  step maps onehot(j0) -> kappa * onehot(j0) (kappa = exp(T[j0,j0])), a
  known scalar correction. The first real step's emission is pre-adjusted
  by expT[START,:]/expT[j0,:] so the chain seeded with onehot(j0)
  reproduces the true alpha recurrence, and every sequence's needed state
  (alpha at len-1) is the final state.

  The time axis is cut into C = 64 chunks of K = 8 steps. Chunks made
  entirely of pads act as known scalars on onehot(j0) and are skipped.
  Each remaining (seq, chunk) pair is an independent "cell" computing the
  8-step 32x32 chunk-product matrix. The host bin-packs all real cells
  (~2200 of 4096) onto 8 cores x 256 device cells; overflow is computed on
  the host in fp64 (a few percent for typical length mixes). Cells pack
  4-to-a-partition-block (lane l = 0..3) x free groups; one 128x128
  block-diagonal expT weight serves every matmul. Per step: one matmul
  (bf16, N=512) + one PSUM->SBUF move (ACT copy or DVE-fused multiply) +
  one broadcast emission multiply (DVE bf16 2x). Four [128, 512] batches
  pipeline across PE/ACT/DVE. Step 0 of each cell is diag(e) @ T' (pure
  input data): batch 0's is precomputed on the host and shipped in the
  first DMA; batches 1-3 compute it with one DVE multiply from a broadcast
  T' tile. The final step's emission multiply is deferred to the host (the
  device ships the raw step-7 matmul result), shortening the output tail.
  The host combines each sequence's chunk matrices in order (fp64
  matvecs), applies the kappa correction and final logsumexp; the
  gold-path score is tiny table gathers on host.
"""

import numpy as np
import ml_dtypes

START_IDX = 29
END_IDX = 30
PAD_IDX = 31

B, S, L = 64, 512, 32
NCORES = 8
J0 = 0              # anchor state for pad steps
KK = 8              # steps per chunk
C = S // KK         # chunks per sequence
NB = 4              # batch pipelines per core
NGB = (2, 2, 2, 2)  # g_hi count per batch
OFF = (0, 2, 4, 6)  # prefix offsets of NGB
NGH = sum(NGB)                    # 8
NCD = 4 * NGH                     # device chunk-slots: c_dev in [0, 32)
CELLS = NCD * 8                   # 256 cells per core
EMCOLS = 8 * KK * NCD             # 2048
BF = ml_dtypes.bfloat16

# (h, s) layer-batches whose PSUM read fuses into the DVE multiply
# (1x from PSUM) instead of ACT copy + 2x DVE multiply; balances ACT vs DVE.
FUSED = {(0, 2), (0, 5), (1, 1), (1, 4), (2, 3), (2, 6)}

_nc = None


def _build_nc():
    import concourse.bacc as bacc
    import concourse.bass as bass
    import concourse.mybir as mybir
    from concourse import tile

    bf = mybir.dt.bfloat16
    f32 = mybir.dt.float32
    nc = bacc.Bacc(None, target_bir_lowering=False)

    inp_in = nc.declare_dram_parameter("inp",
                                       (128, 384 + 256 * NGB[0] + EMCOLS),
                                       bf, isOutput=False)
    am_out = nc.declare_dram_parameter("am", (128, EMCOLS), bf, isOutput=True)

    with tile.TileContext(nc) as tc:
        with (
            tc.tile_pool(name="st", bufs=1) as st,
            tc.tile_pool(name="ps", bufs=1, space=bass.MemorySpace.PSUM) as psp,
        ):
            X00 = 256 * NGB[0]
            W0 = 384 + X00 + 256 * NGB[0]
            E1 = 256 * NGB[1]
            E23 = 256 * (NGB[2] + NGB[3])
            WEM0 = st.tile([128, W0], bf)        # [wt | x0_b0 | em batch0]
            EMR1 = st.tile([128, E1], bf)        # em batch 1
            EMR2 = st.tile([128, E23], bf)       # em batches 2..3
            X = [st.tile([128, 256 * NGB[h]], bf, name=f"X{h}")
                 for h in range(NB)]
            SS = [st.tile([128, 256 * NGB[h]], bf, name=f"SS{h}")
                  for h in range(NB)]
            PS = [psp.tile([128, 256 * NGB[h]], f32, name=f"PS{h}")
                  for h in range(NB)]

            nc.sync.dma_start(WEM0[:], inp_in[:, 0:W0])
            nc.scalar.dma_start(EMR1[:], inp_in[:, W0:W0 + E1])
            nc.sync.dma_start(EMR2[:], inp_in[:, W0 + E1:W0 + E1 + E23])

            WT = WEM0[:, 0:384]
            TBD = WEM0[:, 0:128]
            # TP8[32l+i, j*8+gl] = T'[i, j], replicated over lanes and gl
            em_src = [WEM0[:, 384 + X00:W0], EMR1[:],
                      EMR2[:, 0:256 * NGB[2]],
                      EMR2[:, 256 * NGB[2]:E23]]
            em3 = [
                em_src[h].rearrange("p (blk c) -> p blk c", c=256)
                for h in range(NB)
            ]

            def em_view(h, s):
                v = em3[h][:, 0:NGB[h], 8 * s:8 * s + 8]  # (128, NGB, 8)
                return v[:, None, :, :].broadcast_to([128, 32, NGB[h], 8])

            def tp_view(h):
                v = WT[:, 128:384].rearrange("p (j gl) -> p j gl", j=32)
                return v[:, :, None, :].broadcast_to([128, 32, NGB[h], 8])

            x3 = [
                X[h][:].rearrange("p (j gh gl) -> p j gh gl", j=32, gh=NGB[h])
                for h in range(NB)
            ]
            s3 = [
                SS[h][:].rearrange("p (j gh gl) -> p j gh gl", j=32, gh=NGB[h])
                for h in range(NB)
            ]
            p3 = [
                PS[h][:].rearrange("p (j gh gl) -> p j gh gl", j=32, gh=NGB[h])
                for h in range(NB)
            ]

            # layer 0: A_1 = diag(e_0) @ T'  (chain seeds at I).
            # batch 0's layer-0 state is precomputed on host inside inp.
            nc.vector.tensor_copy(X[0][:], WEM0[:, 384:384 + X00])
            for h in range(1, NB):
                nc.vector.tensor_mul(x3[h], tp_view(h), em_view(h, 0))
            for s in range(1, KK):
                for h in range(NB):
                    fd = 256 * NGB[h]
                    for n0 in range(0, fd, 512):
                        n1 = min(n0 + 512, fd)
                        nc.tensor.matmul(
                            PS[h][:, n0:n1], TBD, X[h][:, n0:n1],
                            start=True, stop=True,
                        )
                    if s == KK - 1:
                        # final move only; host applies the last emission.
                        # batches 0/2 move on ACT, 1/3 on DVE (tail overlap)
                        if h % 2 == 0:
                            nc.scalar.copy(SS[h][:], PS[h][:])
                        else:
                            nc.vector.tensor_copy(SS[h][:], PS[h][:])
                        lo = 256 * OFF[h]
                        eng = nc.sync if h % 2 == 0 else nc.scalar
                        eng.dma_start(am_out[:, lo:lo + 256 * NGB[h]],
                                      SS[h][:])
                    elif (h, s) in FUSED:
                        nc.vector.tensor_mul(x3[h], p3[h], em_view(h, s))
                    else:
                        nc.scalar.copy(SS[h][:], PS[h][:])
                        nc.vector.tensor_mul(x3[h], s3[h], em_view(h, s))

    nc.compile()
    return nc


def _labeled_score(lstm_scores, word_seq_lens, tags, mask, transition):
    b_idx = np.arange(B)
    t0 = tags[:, 0]
    begin = transition[START_IDX, t0].astype(np.float64) + lstm_scores[b_idx, 0, t0]
    prev, curt = tags[:, :-1], tags[:, 1:]
    trans_mid = transition[prev, curt].astype(np.float64)
    em_mid = np.take_along_axis(lstm_scores[:, 1:, :], curt[..., None], axis=2)[..., 0]
    mid = np.where(mask[:, 1:], trans_mid + em_mid, 0.0)
    end_ids = tags[b_idx, word_seq_lens - 1]
    end_sc = transition[end_ids, END_IDX].astype(np.float64)
    return begin.sum() + end_sc.sum() + mid.sum()


def _plan(word_seq_lens):
    """Assign sequences to cores and (seq, chunk) pairs to device cells."""
    lens = word_seq_lens.astype(np.int64)
    shifts = S - lens
    skips = shifts // KK                # leading all-pad chunks
    nreal = C - skips                   # non-pad chunks per seq
    order = np.argsort(-nreal, kind="stable")
    core_seqs = [[] for _ in range(NCORES)]
    sums = np.zeros(NCORES, dtype=np.int64)
    for b in order:
        cg = int(np.argmin(sums))
        core_seqs[cg].append(int(b))
        sums[cg] += nreal[b]
    core_cells = []
    overflow = []
    for cg in range(NCORES):
        cells = []
        for b in core_seqs[cg]:
            for k in range(int(nreal[b])):
                if len(cells) < CELLS:
                    cells.append((b, k))
                else:
                    overflow.append((b, k))
        core_cells.append(cells)
    return core_cells, skips, nreal, overflow


def _prep_inputs(lstm_scores, word_seq_lens, transition):
    """Exp-space right-aligned emissions, packed into device cells (bf16)."""
    T = transition.astype(np.float64)
    expT = np.exp(T)
    Tp = expT.T  # T'[i, j] = expT[j, i]

    lens = word_seq_lens.astype(np.int64)
    shifts = S - lens
    EMfull = np.zeros((B, S, L))
    tau = np.arange(S)[None, :]
    EMfull[:, :, J0] = np.where(tau < shifts[:, None], 1.0, 0.0)
    e0 = np.exp(lstm_scores[:, 0, :].astype(np.float64) + T[START_IDX] - T[J0])
    e0[:, START_IDX] = 0.0
    e0[:, PAD_IDX] = 0.0
    real = np.exp(lstm_scores.astype(np.float64))
    for b in range(B):
        sh = int(shifts[b])
        EMfull[b, sh] = e0[b]
        if sh + 1 < S:
            EMfull[b, sh + 1:] = real[b, 1:S - sh]

    wt = np.zeros((128, 384))
    for l in range(4):
        wt[32 * l:32 * l + 32, 32 * l:32 * l + 32] = expT
    wt[:, 128:384] = np.tile(np.repeat(Tp, 8, axis=1), (4, 1))
    wt = wt.astype(BF)

    core_cells, skips, nreal, overflow = _plan(word_seq_lens)

    ems = []
    e7s = []    # per-core last-step emissions per cell, fp64 [NCD, 8, L]
    for cg in range(NCORES):
        cells = core_cells[cg]
        B_idx = np.zeros((NCD, 8), dtype=np.int64)
        Ct_idx = np.zeros((NCD, 8), dtype=np.int64)
        dummy = np.ones((NCD, 8), dtype=bool)
        for e, (b, k) in enumerate(cells):
            cd, sl = e // 8, e % 8
            B_idx[cd, sl] = b
            Ct_idx[cd, sl] = skips[b] + k
            dummy[cd, sl] = False
        tsel = (KK * Ct_idx[:, None, :] + np.arange(KK)[None, :, None])
        g = EMfull[B_idx[:, None, :], tsel, :]        # (NCD, KK, 8, L)
        oh = np.zeros(L)
        oh[J0] = 1.0
        g[dummy[:, None, :].repeat(KK, 1)] = oh
        e7s.append(g[:, KK - 1, :, :].copy())         # (NCD, 8, L)
        em3d = np.ascontiguousarray(g.transpose(3, 0, 1, 2)).reshape(
            L, NCD * KK * 8)                          # [i, 8*(KK*cd+s)+slot]
        em_rep = np.zeros((128, EMCOLS))
        for l in range(4):
            w = EMCOLS - 64 * l
            em_rep[32 * l:32 * l + 32, :w] = em3d[:, 64 * l:]
        # batch-0 layer-0 state: x0[32l+i, j*8*NGB0 + gh*8 + gl]
        #   = Tp[i, j] * e_first[i, cdev, gl],  cdev = l + 4*(OFF0+gh)
        e_first = em3d.reshape(L, NCD, KK, 8)[:, :, 0, :]
        ngb0 = NGB[0]
        x00 = np.zeros((128, 256 * ngb0))
        jj = np.arange(32)[:, None]
        gll = np.arange(8)[None, :]
        for gh in range(ngb0):
            cols = (jj * 8 * ngb0 + gh * 8 + gll).reshape(-1)
            for l in range(4):
                cdev = l + 4 * (OFF[0] + gh)
                vals = Tp[:, :, None] * e_first[:, cdev, :][:, None, :]
                x00[32 * l:32 * l + 32][:, cols] = vals.reshape(L, -1)
        ems.append(np.concatenate(
            [wt.astype(np.float64), x00, em_rep], axis=1).astype(BF))

    return ems, wt, core_cells, skips, nreal, overflow, EMfull, Tp, e7s


def _device_emulate(ems, wt):
    """Numpy emulation of the device kernel (bf16), for validation."""
    outs = []
    tp8 = wt[:, 128:384].astype(np.float64)
    x00w = 256 * NGB[0]
    for cg in range(NCORES):
        full = ems[cg].astype(np.float64)
        x00 = full[:, 384:384 + x00w]
        em = full[:, 384 + x00w:]
        res = np.zeros((128, EMCOLS))
        for h in range(NB):
            lo, hi = 256 * OFF[h], 256 * (OFF[h] + NGB[h])
            EMh = em[:, lo:hi]
            ngb = NGB[h]

            def emul(s):
                ev = EMh.reshape(128, ngb, 256)[:, :, 8 * s:8 * s + 8]
                return np.repeat(ev[:, None, :, :], 32, axis=1).reshape(128, -1)

            if h == 0:
                Xh = x00.copy()
            else:
                tpv = np.repeat(
                    tp8.reshape(128, 32, 8)[:, :, None, :], ngb, axis=2
                ).reshape(128, -1)
                Xh = np.asarray(tpv * emul(0), dtype=BF).astype(np.float64)
            for s in range(1, KK):
                q = np.zeros_like(Xh)
                for l in range(4):
                    tl = wt[32 * l:32 * l + 32, 32 * l:32 * l + 32] \
                        .astype(np.float64)
                    q[32 * l:32 * l + 32] = tl.T @ Xh[32 * l:32 * l + 32]
                if s == KK - 1:
                    Xh = np.asarray(q, dtype=BF).astype(np.float64)
                else:
                    Xh = np.asarray(q * emul(s), dtype=BF).astype(np.float64)
            res[:, lo:hi] = Xh
        outs.append(res)
    return outs


def _combine(am_list, core_cells, skips, nreal, overflow, EMfull, Tp, e7s,
             word_seq_lens, transition):
    T = transition.astype(np.float64)
    t_end = T[:, END_IDX]
    kappa = T[J0, J0]
    lens = word_seq_lens.astype(np.int64)

    Aof = {}
    for cg in range(NCORES):
        am = np.asarray(am_list[cg]).astype(np.float64)
        e7 = e7s[cg]
        for e, (b, k) in enumerate(core_cells[cg]):
            cd, sl = e // 8, e % 8
            l, ghg = cd % 4, cd // 4
            h = next(hh for hh in range(NB)
                     if OFF[hh] <= ghg < OFF[hh] + NGB[hh])
            gh = ghg - OFF[h]
            cols = (256 * OFF[h] + np.arange(32) * 8 * NGB[h] + gh * 8 + sl)
            # device shipped raw step-7 matmul result; apply last emission
            Aof[(b, k)] = am[32 * l:32 * l + 32][:, cols] * e7[cd, sl][:, None]
    for (b, k) in overflow:
        c_real = int(skips[b] + k)
        M = EMfull[b, KK * c_real][:, None] * Tp
        for s in range(1, KK):
            M = EMfull[b, KK * c_real + s][:, None] * (Tp @ M)
        Aof[(b, k)] = M

    unlabeled = 0.0
    for b in range(B):
        v = np.zeros(L)
        v[J0] = 1.0
        logacc = 0.0
        for k in range(int(nreal[b])):
            v = Aof[(b, k)] @ v
            m = v.max()
            logacc += np.log(m)
            v = v / m
        sh = int(S - lens[b] - skips[b] * KK)
        with np.errstate(divide="ignore"):
            la = np.log(v) + logacc - sh * kappa + t_end
        mm = la.max()
        unlabeled += mm + np.log(np.exp(la - mm).sum())
    return unlabeled


def kernel(lstm_scores, word_seq_lens, tags, mask, transition, _emulate=False):
    global _nc
    lstm_scores = np.asarray(lstm_scores, dtype=np.float32)
    word_seq_lens = np.asarray(word_seq_lens).astype(np.int64)
    tags = np.asarray(tags).astype(np.int64)
    mask = np.asarray(mask).astype(bool)
    transition = np.asarray(transition, dtype=np.float32)

    ems, wt, core_cells, skips, nreal, overflow, EMfull, Tp, e7s = \
        _prep_inputs(lstm_scores, word_seq_lens, transition)

    if _emulate:
        am_list = _device_emulate(ems, wt)
    else:
        if _nc is None:
            _nc = _build_nc()
        in_maps = [{"inp": ems[cg]} for cg in range(NCORES)]
        from concourse.bass_utils import run_bass_kernel_spmd
        res = run_bass_kernel_spmd(_nc, in_maps, list(range(NCORES)))
        am_list = [res.results[cg]["am"] for cg in range(NCORES)]

    unlabeled = _combine(am_list, core_cells, skips, nreal, overflow,
                         EMfull, Tp, e7s, word_seq_lens, transition)
    labeled = _labeled_score(lstm_scores, word_seq_lens, tags, mask, transition)
    return (np.float32(unlabeled), np.float32(labeled))


# revision 23
# speedup vs baseline: 1.0456x; 1.0033x over previous
"""Linear-chain CRF loss (forward partition + gold score) on 8 Trainium2 cores.

Strategy (compacted chunked matrix scan):
  The 512-step forward recurrence p' = diag(e_t) @ expT.T @ p (exp space) is
  linear, so alpha at any position is a product of per-step 32x32 factors.
  Sequences are RIGHT-ALIGNED on the host: a sequence of length n gets
  S - n leading "pad" steps whose exp-space emission is onehot(j0); a pad
  step maps onehot(j0) -> kappa * onehot(j0) (kappa = exp(T[j0,j0])), a
  known scalar correction. The first real step's emission is pre-adjusted
  by expT[START,:]/expT[j0,:] so the chain seeded with onehot(j0)
  reproduces the true alpha recurrence, and every sequence's needed state
  (alpha at len-1) is the final state.

  The time axis is cut into C = 64 chunks of K = 8 steps. Chunks made
  entirely of pads act as known scalars on onehot(j0) and are skipped.
  Each remaining (seq, chunk) pair is an independent "cell" computing the
  8-step 32x32 chunk-product matrix. The host bin-packs all real cells
  (~2200 of 4096) onto 8 cores x 256 device cells; overflow is computed on
  the host in fp64 (a few percent for typical length mixes). Cells pack
  4-to-a-partition-block (lane l = 0..3) x free groups; one 128x128
  block-diagonal expT weight serves every matmul. Per step: one matmul
  (bf16, N=512) + one PSUM->SBUF move (ACT copy or DVE-fused multiply) +
  one broadcast emission multiply (DVE bf16 2x). Four [128, 512] batches
  pipeline across PE/ACT/DVE. Step 0 of each cell is diag(e) @ T' (pure
  input data): batch 0's is precomputed on the host and shipped in the
  first DMA; batches 1-3 compute it with one DVE multiply from a broadcast
  T' tile. The final step's emission multiply is deferred to the host (the
  device ships the raw step-7 matmul result), shortening the output tail.
  The host combines each sequence's chunk matrices in order (fp64
  matvecs), applies the kappa correction and final logsumexp; the
  gold-path score is tiny table gathers on host.
"""

import numpy as np
import ml_dtypes

START_IDX = 29
END_IDX = 30
PAD_IDX = 31

B, S, L = 64, 512, 32
NCORES = 8
J0 = 0              # anchor state for pad steps
KK = 8              # steps per chunk
C = S // KK         # chunks per sequence
NB = 4              # batch pipelines per core
NGB = (2, 2, 2, 2)  # g_hi count per batch
OFF = (0, 2, 4, 6)  # prefix offsets of NGB
NGH = sum(NGB)                    # 8
NCD = 4 * NGH                     # device chunk-slots: c_dev in [0, 32)
CELLS = NCD * 8                   # 256 cells per core
EMCOLS = 8 * KK * NCD             # 2048
BF = ml_dtypes.bfloat16

# (h, s) layer-batches whose PSUM read fuses into the DVE multiply
# (1x from PSUM) instead of ACT copy + 2x DVE multiply; balances ACT vs DVE.
FUSED = {(0, 2), (0, 5), (1, 1), (1, 4), (2, 3), (2, 6)}

_nc = None


def _build_nc():
    import concourse.bacc as bacc
    import concourse.bass as bass
    import concourse.mybir as mybir
    from concourse import tile

    bf = mybir.dt.bfloat16
    f32 = mybir.dt.float32
    nc = bacc.Bacc(None, target_bir_lowering=False)

    inp_in = nc.declare_dram_parameter("inp",
                                       (128, 384 + 256 * NGB[0] + EMCOLS),
                                       bf, isOutput=False)
    am_out = nc.declare_dram_parameter("am", (128, EMCOLS), bf, isOutput=True)

    with tile.TileContext(nc) as tc:
        with (
            tc.tile_pool(name="st", bufs=1) as st,
            tc.tile_pool(name="ps", bufs=1, space=bass.MemorySpace.PSUM) as psp,
        ):
            X00 = 256 * NGB[0]
            W0 = 384 + X00 + 256 * NGB[0]
            E1 = 256 * NGB[1]
            E23 = 256 * (NGB[2] + NGB[3])
            WEM0 = st.tile([128, W0], bf)        # [wt | x0_b0 | em batch0]
            EMR1 = st.tile([128, E1], bf)        # em batch 1
            EMR2 = st.tile([128, E23], bf)       # em batches 2..3
            X = [st.tile([128, 256 * NGB[h]], bf, name=f"X{h}")
                 for h in range(NB)]
            SS = [st.tile([128, 256 * NGB[h]], bf, name=f"SS{h}")
                  for h in range(NB)]
            PS = [psp.tile([128, 256 * NGB[h]], f32, name=f"PS{h}")
                  for h in range(NB)]

            nc.sync.dma_start(WEM0[:], inp_in[:, 0:W0])
            nc.scalar.dma_start(EMR1[:], inp_in[:, W0:W0 + E1])
            nc.sync.dma_start(EMR2[:], inp_in[:, W0 + E1:W0 + E1 + E23])

            WT = WEM0[:, 0:384]
            TBD = WEM0[:, 0:128]
            # TP8[32l+i, j*8+gl] = T'[i, j], replicated over lanes and gl
            em_src = [WEM0[:, 384 + X00:W0], EMR1[:],
                      EMR2[:, 0:256 * NGB[2]],
                      EMR2[:, 256 * NGB[2]:E23]]
            em3 = [
                em_src[h].rearrange("p (blk c) -> p blk c", c=256)
                for h in range(NB)
            ]

            def em_view(h, s):
                v = em3[h][:, 0:NGB[h], 8 * s:8 * s + 8]  # (128, NGB, 8)
                return v[:, None, :, :].broadcast_to([128, 32, NGB[h], 8])

            def tp_view(h):
                v = WT[:, 128:384].rearrange("p (j gl) -> p j gl", j=32)
                return v[:, :, None, :].broadcast_to([128, 32, NGB[h], 8])

            x3 = [
                X[h][:].rearrange("p (j gh gl) -> p j gh gl", j=32, gh=NGB[h])
                for h in range(NB)
            ]
            s3 = [
                SS[h][:].rearrange("p (j gh gl) -> p j gh gl", j=32, gh=NGB[h])
                for h in range(NB)
            ]
            p3 = [
                PS[h][:].rearrange("p (j gh gl) -> p j gh gl", j=32, gh=NGB[h])
                for h in range(NB)
            ]

            # layer 0: A_1 = diag(e_0) @ T'  (chain seeds at I).
            # batch 0's layer-0 state is precomputed on host inside inp.
            nc.vector.tensor_copy(X[0][:], WEM0[:, 384:384 + X00])
            for h in range(1, NB):
                nc.vector.tensor_mul(x3[h], tp_view(h), em_view(h, 0))
            for s in range(1, KK):
                for h in range(NB):
                    fd = 256 * NGB[h]
                    for n0 in range(0, fd, 512):
                        n1 = min(n0 + 512, fd)
                        nc.tensor.matmul(
                            PS[h][:, n0:n1], TBD, X[h][:, n0:n1],
                            start=True, stop=True,
                        )
                    if s == KK - 1:
                        # final move only; host applies the last emission.
                        # batches 0/1 move on ACT, 2/3 on DVE (tail overlap)
                        if h < 2:
                            nc.scalar.copy(SS[h][:], PS[h][:])
                        else:
                            nc.vector.tensor_copy(SS[h][:], PS[h][:])
                        lo = 256 * OFF[h]
                        eng = nc.sync if h % 2 == 0 else nc.scalar
                        eng.dma_start(am_out[:, lo:lo + 256 * NGB[h]],
                                      SS[h][:])
                    elif (h, s) in FUSED:
                        nc.vector.tensor_mul(x3[h], p3[h], em_view(h, s))
                    else:
                        nc.scalar.copy(SS[h][:], PS[h][:])
                        nc.vector.tensor_mul(x3[h], s3[h], em_view(h, s))

    nc.compile()
    return nc


def _labeled_score(lstm_scores, word_seq_lens, tags, mask, transition):
    b_idx = np.arange(B)
    t0 = tags[:, 0]
    begin = transition[START_IDX, t0].astype(np.float64) + lstm_scores[b_idx, 0, t0]
    prev, curt = tags[:, :-1], tags[:, 1:]
    trans_mid = transition[prev, curt].astype(np.float64)
    em_mid = np.take_along_axis(lstm_scores[:, 1:, :], curt[..., None], axis=2)[..., 0]
    mid = np.where(mask[:, 1:], trans_mid + em_mid, 0.0)
    end_ids = tags[b_idx, word_seq_lens - 1]
    end_sc = transition[end_ids, END_IDX].astype(np.float64)
    return begin.sum() + end_sc.sum() + mid.sum()


def _plan(word_seq_lens):
    """Assign sequences to cores and (seq, chunk) pairs to device cells."""
    lens = word_seq_lens.astype(np.int64)
    shifts = S - lens
    skips = shifts // KK                # leading all-pad chunks
    nreal = C - skips                   # non-pad chunks per seq
    order = np.argsort(-nreal, kind="stable")
    core_seqs = [[] for _ in range(NCORES)]
    sums = np.zeros(NCORES, dtype=np.int64)
    for b in order:
        cg = int(np.argmin(sums))
        core_seqs[cg].append(int(b))
        sums[cg] += nreal[b]
    core_cells = []
    overflow = []
    for cg in range(NCORES):
        cells = []
        for b in core_seqs[cg]:
            for k in range(int(nreal[b])):
                if len(cells) < CELLS:
                    cells.append((b, k))
                else:
                    overflow.append((b, k))
        core_cells.append(cells)
    return core_cells, skips, nreal, overflow


def _prep_inputs(lstm_scores, word_seq_lens, transition):
    """Exp-space right-aligned emissions, packed into device cells (bf16)."""
    T = transition.astype(np.float64)
    expT = np.exp(T)
    Tp = expT.T  # T'[i, j] = expT[j, i]

    lens = word_seq_lens.astype(np.int64)
    shifts = S - lens
    EMfull = np.zeros((B, S, L))
    tau = np.arange(S)[None, :]
    EMfull[:, :, J0] = np.where(tau < shifts[:, None], 1.0, 0.0)
    e0 = np.exp(lstm_scores[:, 0, :].astype(np.float64) + T[START_IDX] - T[J0])
    e0[:, START_IDX] = 0.0
    e0[:, PAD_IDX] = 0.0
    real = np.exp(lstm_scores.astype(np.float64))
    for b in range(B):
        sh = int(shifts[b])
        EMfull[b, sh] = e0[b]
        if sh + 1 < S:
            EMfull[b, sh + 1:] = real[b, 1:S - sh]

    wt = np.zeros((128, 384))
    for l in range(4):
        wt[32 * l:32 * l + 32, 32 * l:32 * l + 32] = expT
    wt[:, 128:384] = np.tile(np.repeat(Tp, 8, axis=1), (4, 1))
    wt = wt.astype(BF)

    core_cells, skips, nreal, overflow = _plan(word_seq_lens)

    ems = []
    e7s = []    # per-core last-step emissions per cell, fp64 [NCD, 8, L]
    for cg in range(NCORES):
        cells = core_cells[cg]
        B_idx = np.zeros((NCD, 8), dtype=np.int64)
        Ct_idx = np.zeros((NCD, 8), dtype=np.int64)
        dummy = np.ones((NCD, 8), dtype=bool)
        for e, (b, k) in enumerate(cells):
            cd, sl = e // 8, e % 8
            B_idx[cd, sl] = b
            Ct_idx[cd, sl] = skips[b] + k
            dummy[cd, sl] = False
        tsel = (KK * Ct_idx[:, None, :] + np.arange(KK)[None, :, None])
        g = EMfull[B_idx[:, None, :], tsel, :]        # (NCD, KK, 8, L)
        oh = np.zeros(L)
        oh[J0] = 1.0
        g[dummy[:, None, :].repeat(KK, 1)] = oh
        e7s.append(g[:, KK - 1, :, :].copy())         # (NCD, 8, L)
        em3d = np.ascontiguousarray(g.transpose(3, 0, 1, 2)).reshape(
            L, NCD * KK * 8)                          # [i, 8*(KK*cd+s)+slot]
        em_rep = np.zeros((128, EMCOLS))
        for l in range(4):
            w = EMCOLS - 64 * l
            em_rep[32 * l:32 * l + 32, :w] = em3d[:, 64 * l:]
        # batch-0 layer-0 state: x0[32l+i, j*8*NGB0 + gh*8 + gl]
        #   = Tp[i, j] * e_first[i, cdev, gl],  cdev = l + 4*(OFF0+gh)
        e_first = em3d.reshape(L, NCD, KK, 8)[:, :, 0, :]
        ngb0 = NGB[0]
        x00 = np.zeros((128, 256 * ngb0))
        jj = np.arange(32)[:, None]
        gll = np.arange(8)[None, :]
        for gh in range(ngb0):
            cols = (jj * 8 * ngb0 + gh * 8 + gll).reshape(-1)
            for l in range(4):
                cdev = l + 4 * (OFF[0] + gh)
                vals = Tp[:, :, None] * e_first[:, cdev, :][:, None, :]
                x00[32 * l:32 * l + 32][:, cols] = vals.reshape(L, -1)
        ems.append(np.concatenate(
            [wt.astype(np.float64), x00, em_rep], axis=1).astype(BF))

    return ems, wt, core_cells, skips, nreal, overflow, EMfull, Tp, e7s


def _device_emulate(ems, wt):
    """Numpy emulation of the device kernel (bf16), for validation."""
    outs = []
    tp8 = wt[:, 128:384].astype(np.float64)
    x00w = 256 * NGB[0]
    for cg in range(NCORES):
        full = ems[cg].astype(np.float64)
        x00 = full[:, 384:384 + x00w]
        em = full[:, 384 + x00w:]
        res = np.zeros((128, EMCOLS))
        for h in range(NB):
            lo, hi = 256 * OFF[h], 256 * (OFF[h] + NGB[h])
            EMh = em[:, lo:hi]
            ngb = NGB[h]

            def emul(s):
                ev = EMh.reshape(128, ngb, 256)[:, :, 8 * s:8 * s + 8]
                return np.repeat(ev[:, None, :, :], 32, axis=1).reshape(128, -1)

            if h == 0:
                Xh = x00.copy()
            else:
                tpv = np.repeat(
                    tp8.reshape(128, 32, 8)[:, :, None, :], ngb, axis=2
                ).reshape(128, -1)
                Xh = np.asarray(tpv * emul(0), dtype=BF).astype(np.float64)
            for s in range(1, KK):
                q = np.zeros_like(Xh)
                for l in range(4):
                    tl = wt[32 * l:32 * l + 32, 32 * l:32 * l + 32] \
                        .astype(np.float64)
                    q[32 * l:32 * l + 32] = tl.T @ Xh[32 * l:32 * l + 32]
                if s == KK - 1:
                    Xh = np.asarray(q, dtype=BF).astype(np.float64)
                else:
                    Xh = np.asarray(q * emul(s), dtype=BF).astype(np.float64)
            res[:, lo:hi] = Xh
        outs.append(res)
    return outs


def _combine(am_list, core_cells, skips, nreal, overflow, EMfull, Tp, e7s,
             word_seq_lens, transition):
    T = transition.astype(np.float64)
    t_end = T[:, END_IDX]
    kappa = T[J0, J0]
    lens = word_seq_lens.astype(np.int64)

    Aof = {}
    for cg in range(NCORES):
        am = np.asarray(am_list[cg]).astype(np.float64)
        e7 = e7s[cg]
        for e, (b, k) in enumerate(core_cells[cg]):
            cd, sl = e // 8, e % 8
            l, ghg = cd % 4, cd // 4
            h = next(hh for hh in range(NB)
                     if OFF[hh] <= ghg < OFF[hh] + NGB[hh])
            gh = ghg - OFF[h]
            cols = (256 * OFF[h] + np.arange(32) * 8 * NGB[h] + gh * 8 + sl)
            # device shipped raw step-7 matmul result; apply last emission
            Aof[(b, k)] = am[32 * l:32 * l + 32][:, cols] * e7[cd, sl][:, None]
    for (b, k) in overflow:
        c_real = int(skips[b] + k)
        M = EMfull[b, KK * c_real][:, None] * Tp
        for s in range(1, KK):
            M = EMfull[b, KK * c_real + s][:, None] * (Tp @ M)
        Aof[(b, k)] = M

    unlabeled = 0.0
    for b in range(B):
        v = np.zeros(L)
        v[J0] = 1.0
        logacc = 0.0
        for k in range(int(nreal[b])):
            v = Aof[(b, k)] @ v
            m = v.max()
            logacc += np.log(m)
            v = v / m
        sh = int(S - lens[b] - skips[b] * KK)
        with np.errstate(divide="ignore"):
            la = np.log(v) + logacc - sh * kappa + t_end
        mm = la.max()
        unlabeled += mm + np.log(np.exp(la - mm).sum())
    return unlabeled


def kernel(lstm_scores, word_seq_lens, tags, mask, transition, _emulate=False):
    global _nc
    lstm_scores = np.asarray(lstm_scores, dtype=np.float32)
    word_seq_lens = np.asarray(word_seq_lens).astype(np.int64)
    tags = np.asarray(tags).astype(np.int64)
    mask = np.asarray(mask).astype(bool)
    transition = np.asarray(transition, dtype=np.float32)

    ems, wt, core_cells, skips, nreal, overflow, EMfull, Tp, e7s = \
        _prep_inputs(lstm_scores, word_seq_lens, transition)

    if _emulate:
        am_list = _device_emulate(ems, wt)
    else:
        if _nc is None:
            _nc = _build_nc()
        in_maps = [{"inp": ems[cg]} for cg in range(NCORES)]
        from concourse.bass_utils import run_bass_kernel_spmd
        res = run_bass_kernel_spmd(_nc, in_maps, list(range(NCORES)))
        am_list = [res.results[cg]["am"] for cg in range(NCORES)]

    unlabeled = _combine(am_list, core_cells, skips, nreal, overflow,
                         EMfull, Tp, e7s, word_seq_lens, transition)
    labeled = _labeled_score(lstm_scores, word_seq_lens, tags, mask, transition)
    return (np.float32(unlabeled), np.float32(labeled))
